# revision 1
# baseline (speedup 1.0000x reference)
"""Trainium2 Bass kernel for nn_ContrastiveLoss (exp-cosine ranking loss).

Math: sort rows of output1 by descending ranking (stable). With
e_b[i] = exp(cos_sim(x_sorted[i], o_b)) for b in {2,3} and suffix sums
suf_b(i) = sum_{j>=i} e_b[j], the reference loss equals

    loss = N*(log T2 + log T3) - sum_i log suf2(i) - sum_i log suf3(i)

where T_b = suf_b(0) is the global total.  Sharding: host sorts by
ranking (the sort defines the shard boundaries, i.e. shards are
rank-contiguous) and feeds rows in ASCENDING rank order so forward
cumsums on-device are exactly the suffix sums of the reference order.
Each core gets its 8192-row shard in TRANSPOSED layout [512, 8192]
(a pure host-side relayout of the same f32 data) so the tensor engine
can do the heavy lifting:

  PE:   dots d2/d3 via matmul (xT chunk stationary, [o2,o3] moving),
        plus transposes of each xT chunk back to row-major in PSUM
  ACT:  Square+accumulate on the PSUM row-major tiles -> row |x|^2
  DVE:  only small tail work (exp-cosine prep, scans, copies)

The per-shard scan machinery runs before/during the AllGather wait; the
global base lands as the per-partition bias of the final Ln activation,
and a second tiny AllGather makes every core emit the same final scalar.
"""

import numpy as np

N, D = 65536, 512
NCORES = 8
SH = N // NCORES            # 8192 rows per core
TPC = SH // 128             # 64 row-tiles of 128 per core
NCH = D // 128              # 4 contraction chunks of 128
RBLK = 512                  # rows per DMA block (1MB transfers)
NBLK = SH // RBLK           # 16 DMA blocks
GPB = RBLK // 128           # 4 row-groups per block

_compiled_nc = None


def _body(tc, mybir, masks, xs, o2b_d, o3b_d, o23_d, mlt, loss_out):
    """Emit the per-core Tile kernel. All args are bass.APs of DRAM tensors."""
    nc = tc.nc
    f32 = mybir.dt.float32
    OP = mybir.AluOpType
    AF = mybir.ActivationFunctionType
    AX = mybir.AxisListType

    with (
        tc.tile_pool(name="const", bufs=1) as constp,
        tc.tile_pool(name="xin", bufs=6) as xinp,
        tc.tile_pool(name="scr", bufs=2) as scrp,
        tc.tile_pool(name="stats", bufs=1) as statsp,
        tc.tile_pool(name="small", bufs=1) as smallp,
        tc.tile_pool(name="psum", bufs=1, space="PSUM") as psump,
        tc.tile_pool(name="dram", bufs=1, space="DRAM") as dramp,
    ):
        # ---- constants (small queue: gpsimd; bulk stream uses sync) ----
        o2b = constp.tile([128, D], f32)
        nc.gpsimd.dma_start(o2b[:], o2b_d)
        o3b = constp.tile([128, D], f32)
        nc.gpsimd.dma_start(o3b[:], o3b_d)
        o23 = constp.tile([128, NCH, 2], f32)
        nc.gpsimd.dma_start(o23[:], o23_d)
        mltt = constp.tile([8, 128], f32)
        nc.gpsimd.dma_start(mltt[:], mlt)
        ident = constp.tile([128, 128], f32)
        masks.make_identity(nc, ident[:])
        ones128 = constp.tile([128, 1], f32)
        nc.vector.memset(ones128[:], 1.0)

        # 1/||o2||, 1/||o3|| replicated on every partition
        sco = scrp.tile([128, D], f32, tag="actscr")
        so2 = smallp.tile([128, 1], f32)
        nc.scalar.activation(sco[:], o2b[:], AF.Square, accum_out=so2[:])
        n2b = smallp.tile([128, 1], f32)
        nc.scalar.activation(n2b[:], so2[:], AF.Sqrt)
        invn2b = smallp.tile([128, 1], f32)
        nc.vector.reciprocal(invn2b[:], n2b[:])
        sco2 = scrp.tile([128, D], f32, tag="actscr")
        so3 = smallp.tile([128, 1], f32)
        nc.scalar.activation(sco2[:], o3b[:], AF.Square, accum_out=so3[:])
        n3b = smallp.tile([128, 1], f32)
        nc.scalar.activation(n3b[:], so3[:], AF.Sqrt)
        invn3b = smallp.tile([128, 1], f32)
        nc.vector.reciprocal(invn3b[:], n3b[:])

        # ---- phase 1: dots (PE) + row sum-of-squares (PE transpose + ACT) ----
        d23all = statsp.tile([128, TPC, 2], f32)
        ssall = statsp.tile([128, TPC], f32)

        # xs is xT [D, SH]; tile (p=d-in-chunk, c=chunk, r=row-in-block)
        xv = xs.rearrange("(c p) (g r) -> g p c r", p=128, g=NBLK)
        for g in range(NBLK):
            xt = xinp.tile([128, NCH, RBLK], f32)
            nc.sync.dma_start(xt[:], xv[g])
            for rg in range(GPB):
                t = g * GPB + rg
                rows = slice(rg * 128, (rg + 1) * 128)
                dots_ps = psump.tile([128, 2], f32, tag="dots", bufs=2)
                xrm_ps = psump.tile([128, D], f32, tag="xrm", bufs=2)
                for c in range(NCH):
                    nc.tensor.matmul(
                        dots_ps[:], xt[:, c, rows], o23[:, c, :],
                        start=(c == 0), stop=(c == NCH - 1))
                for c in range(NCH):
                    nc.tensor.transpose(
                        xrm_ps[:, c * 128 : (c + 1) * 128], xt[:, c, rows],
                        ident[:])
                s3 = scrp.tile([128, D], f32, tag="actscr")
                nc.scalar.activation(
                    s3[:], xrm_ps[:], AF.Square, accum_out=ssall[:, t : t + 1])
                nc.vector.tensor_copy(d23all[:, t, :], dots_ps[:])

        # ---- phase 2: exp-cosines ----
        nrm = statsp.tile([128, TPC], f32)
        nc.scalar.activation(nrm[:], ssall[:], AF.Sqrt)
        rs = statsp.tile([128, TPC], f32)
        nc.vector.reciprocal(rs[:], nrm[:])
        t2 = statsp.tile([128, TPC], f32)
        nc.vector.tensor_tensor(out=t2[:], in0=d23all[:, :, 0], in1=rs[:], op=OP.mult)
        t3 = statsp.tile([128, TPC], f32)
        nc.vector.tensor_tensor(out=t3[:], in0=d23all[:, :, 1], in1=rs[:], op=OP.mult)
        # eall[:, 0:64] = e2 per (row p, tile t); eall[:, 64:128] = e3
        eall = statsp.tile([128, 2 * TPC], f32)
        nc.scalar.activation(eall[:, 0:TPC], t2[:], AF.Exp, scale=invn2b[:])
        nc.scalar.activation(eall[:, TPC:], t3[:], AF.Exp, scale=invn3b[:])

        # ---- phase 3a: local totals -> post the AllGather as early as possible
        # per-(branch,tile) totals, row layout: totr[0, q] = sum_p eall[p, q]
        totr_ps = psump.tile([1, 128], f32, tag="tailshort", bufs=2)
        nc.tensor.matmul(totr_ps[:], ones128[:], eall[:], start=True, stop=True)
        totr = smallp.tile([1, 128], f32)
        nc.vector.tensor_copy(totr[:], totr_ps[:])
        tl = smallp.tile([1, 2], f32)
        nc.vector.tensor_reduce(out=tl[:, 0:1], in_=totr[:, 0:TPC], axis=AX.X, op=OP.add)
        nc.vector.tensor_reduce(out=tl[:, 1:2], in_=totr[:, TPC:], axis=AX.X, op=OP.add)
        cc_in = dramp.tile([1, 2], f32)
        cc_out = dramp.tile([8, 2], f32, addr_space="Shared")
        nc.sync.dma_start(cc_in[:], tl[:])
        nc.gpsimd.collective_compute(
            "AllGather", OP.bypass, replica_groups=[list(range(NCORES))],
            ins=[cc_in.opt()], outs=[cc_out.opt()])

        # ---- phase 3b: shard-local scans (overlap the AllGather skew wait)
        # transpose -> eT[q, p] with q = branch*64 + t
        eT_ps = psump.tile([128, 128], f32, tag="tailshort", bufs=2)
        nc.tensor.transpose(eT_ps[:], eall[:], ident[:])
        eT = statsp.tile([128, 128], f32)
        nc.scalar.copy(eT[:], eT_ps[:])
        # shifted (exclusive) tile totals, local only
        sh = smallp.tile([1, 128], f32)
        nc.vector.memset(sh[:, 0:1], 0.0)
        nc.vector.memset(sh[:, TPC : TPC + 1], 0.0)
        nc.vector.tensor_copy(sh[:, 1:TPC], totr[:, 0 : TPC - 1])
        nc.vector.tensor_copy(sh[:, TPC + 1 :], totr[:, TPC : 2 * TPC - 1])
        baser = smallp.tile([1, 128], f32)
        nc.vector.tensor_tensor_scan(
            out=baser[:, 0:TPC], data0=sh[:, 0:TPC], data1=sh[:, 0:TPC],
            initial=0.0, op0=OP.add, op1=OP.bypass)
        nc.vector.tensor_tensor_scan(
            out=baser[:, TPC:], data0=sh[:, TPC:], data1=sh[:, TPC:],
            initial=0.0, op0=OP.add, op1=OP.bypass)
        # move per-tile bases onto partitions: basec[q, 0] = baser[0, q]
        basec = smallp.tile([128, 1], f32)
        nc.sync.dma_start(basec[:], baser[:])
        # inclusive scan within each tile (along p) seeded by the local base:
        # sufl[q, p] = local suffix sums (missing only the global core base)
        sufl = statsp.tile([128, 128], f32)
        nc.vector.tensor_tensor_scan(
            out=sufl[:], data0=eT[:], data1=eT[:], initial=basec[:],
            op0=OP.add, op1=OP.bypass)

        # ---- phase 3c: consume the AllGather ----
        ag = smallp.tile([8, 2], f32)
        nc.sync.dma_start(ag[:], cc_out[:])
        # per-partition global bases: gb_ps[q, b] = sum_{c < my_core} tot_b[c]
        gb_ps = psump.tile([128, 2], f32, tag="gbps")
        nc.tensor.matmul(gb_ps[:], mltt[:], ag[:], start=True, stop=True)
        tg_ps = psump.tile([1, 2], f32, tag="tgps")
        nc.tensor.matmul(tg_ps[:], ones128[0:8, :], ag[:], start=True, stop=True)
        gb = smallp.tile([128, 2], f32)
        nc.vector.tensor_copy(gb[:], gb_ps[:])

        # ---- phase 4: log-reduction (global base folded into Ln bias) ----
        lnscr = statsp.tile([128, 128], f32)
        lnacc = smallp.tile([128, 1], f32)
        nc.scalar.activation(lnscr[0:TPC, :], sufl[0:TPC, :], AF.Ln,
                             bias=gb[0:TPC, 0:1], accum_out=lnacc[0:TPC, :])
        nc.scalar.activation(lnscr[TPC:, :], sufl[TPC:, :], AF.Ln,
                             bias=gb[TPC:, 1:2], accum_out=lnacc[TPC:, :])
        part_ps = psump.tile([1, 1], f32, tag="tailshort", bufs=2)
        nc.tensor.matmul(part_ps[:], ones128[:], lnacc[:], start=True, stop=True)
        parts = smallp.tile([1, 1], f32)
        nc.vector.tensor_copy(parts[:], part_ps[:])

        # AllGather the per-core log-sums; N*(log T2 + log T3) overlaps it
        cc2_in = dramp.tile([1, 1], f32)
        cc2_out = dramp.tile([8, 1], f32, addr_space="Shared")
        nc.sync.dma_start(cc2_in[:], parts[:])
        nc.gpsimd.collective_compute(
            "AllGather", OP.bypass, replica_groups=[list(range(NCORES))],
            ins=[cc2_in.opt()], outs=[cc2_out.opt()])
        lt = smallp.tile([1, 2], f32)
        nc.scalar.activation(lt[:], tg_ps[:], AF.Ln)
        lts = smallp.tile([1, 1], f32)
        nc.vector.tensor_reduce(out=lts[:], in_=lt[:], axis=AX.X, op=OP.add)
        f1 = smallp.tile([1, 1], f32)
        nc.scalar.mul(f1[:], lts[:], float(N))
        # final = N*(log T2 + log T3) - sum over cores of log-sums
        agp = smallp.tile([8, 1], f32)
        nc.sync.dma_start(agp[:], cc2_out[:])
        s_ps = psump.tile([1, 1], f32, tag="tailshort", bufs=2)
        nc.tensor.matmul(s_ps[:], ones128[0:8, :], agp[:], start=True, stop=True)
        fin = smallp.tile([1, 1], f32)
        nc.vector.tensor_tensor(out=fin[:], in0=f1[:], in1=s_ps[:], op=OP.subtract)
        nc.sync.dma_start(loss_out[:], fin[:])


def build_nc():
    """Build + compile the SPMD Bass program (cached)."""
    global _compiled_nc
    if _compiled_nc is not None:
        return _compiled_nc
    import concourse.bacc as bacc
    import concourse.mybir as mybir
    from concourse import masks, tile

    f32 = mybir.dt.float32
    nc = bacc.Bacc("TRN2", target_bir_lowering=False, debug=False,
                   num_devices=NCORES)
    xs = nc.dram_tensor("xs", [D, SH], f32, kind="ExternalInput")
    o2b = nc.dram_tensor("o2b", [128, D], f32, kind="ExternalInput")
    o3b = nc.dram_tensor("o3b", [128, D], f32, kind="ExternalInput")
    o23 = nc.dram_tensor("o23", [128, NCH, 2], f32, kind="ExternalInput")
    mlt = nc.dram_tensor("mlt", [8, 128], f32, kind="ExternalInput")
    loss = nc.dram_tensor("loss", [1, 1], f32, kind="ExternalOutput")

    with tile.TileContext(nc) as tc:
        _body(tc, mybir, masks, xs.ap(), o2b.ap(), o3b.ap(), o23.ap(),
              mlt.ap(), loss.ap())
    nc.compile()
    _compiled_nc = nc
    return nc


def make_in_maps(output1, output2, output3, ranking):
    """Host-side shard: sort rows by descending ranking (stable, matching
    jnp.argsort(-ranking)), feed in reversed (ascending) order so forward
    cumsums on-device are the reference's suffix sums, and lay each shard
    out transposed [D, SH] for the tensor engine."""
    ranking = np.asarray(ranking, dtype=np.float32)
    order = np.argsort(-ranking, kind="stable")
    rho = order[::-1]
    xs_full = np.asarray(output1, dtype=np.float32)[rho]
    o2 = np.asarray(output2, dtype=np.float32).reshape(D)
    o3 = np.asarray(output3, dtype=np.float32).reshape(D)
    o2b = np.ascontiguousarray(np.broadcast_to(o2[None, :], (128, D)))
    o3b = np.ascontiguousarray(np.broadcast_to(o3[None, :], (128, D)))
    o23 = np.empty((128, NCH, 2), np.float32)
    o23[:, :, 0] = o2.reshape(NCH, 128).T
    o23[:, :, 1] = o3.reshape(NCH, 128).T
    in_maps = []
    for c in range(NCORES):
        mlt = np.zeros((8, 128), np.float32)
        mlt[:c] = 1.0
        in_maps.append({
            "xs": np.ascontiguousarray(xs_full[c * SH : (c + 1) * SH].T),
            "o2b": o2b, "o3b": o3b, "o23": o23, "mlt": mlt,
        })
    return in_maps


def kernel(output1, output2, output3, ranking):
    from concourse.bass_utils import run_bass_kernel_spmd

    nc = build_nc()
    in_maps = make_in_maps(output1, output2, output3, ranking)
    res = run_bass_kernel_spmd(nc, in_maps, core_ids=list(range(NCORES)))
    out = res.results[0]["loss"]
    return np.asarray(out, dtype=np.float32).reshape(())



# revision 16
# speedup vs baseline: 2.4232x; 2.4232x over previous
"""Trainium2 Bass kernel for nn_ContrastiveLoss (exp-cosine ranking loss).

Math: sort rows of output1 by descending ranking (stable). With
e_b[i] = exp(cos_sim(x_sorted[i], o_b)) for b in {2,3} and suffix sums
suf_b(i) = sum_{j>=i} e_b[j], the reference loss equals

    loss = N*(log T2 + log T3) - sum_i log suf2(i) - sum_i log suf3(i)

where T_b = suf_b(0) is the global total.  Sharding: host sorts by
ranking (shards are rank-contiguous) and feeds rows in ASCENDING rank
order so forward cumsums on-device are exactly the suffix sums of the
reference order.  Each core gets its 8192-row shard TRANSPOSED
[512, 8192] in bf16.

Per 512-row block (16 blocks/core), everything is *streamed* through
the PE array (no transposes, no 128-col weight loads):

  dots:  o23 chunk [128,2] stationary, xT chunk [128,512] moving
         -> PSUM [2,512] accumulated over the 4 K-chunks
  norms: ACT squares the xT tile (bf16, 2 elem/cycle); ones [128,2]
         stationary, squares moving -> PSUM [2,512] = row |x|^2 (dup'd)
  DVE:   scatters both PSUM tiles into [32,512] stats tiles
         (partition 2g=branch2, 2g+1=branch3 for block g)

Bulk DMA alternates between the sync (HWDGE) and gpsimd (SWDGE) queues
so two queues stream HBM concurrently.  The tail (sqrt/recip/exp,
per-block scan seeded by a strict-lower-triangular matmul, one tiny
AllGather for the cross-core base, Ln+accumulate) runs once on
[32,512].  Each core outputs (tot2, tot3, sum-of-logs); the host sums
8 of each and forms  N*(log T2 + log T3) - sum(partials).
"""

import numpy as np

N, D = 65536, 512
NCORES = 8
SH = N // NCORES            # 8192 rows per core
NCH = D // 128              # 4 contraction chunks of 128
RBLK = 512                  # rows per block
NBLK = SH // RBLK           # 16 blocks
NP32 = 2 * NBLK             # 32 stats partitions: (block g, branch b) -> 2g+b

_compiled_nc = None


def _body(tc, mybir, xs, o23w_d, onesw_d, o23i_d, l32_d, mlt16_d, sel32_d,
          fin_out):
    nc = tc.nc
    f32 = mybir.dt.float32
    bf16 = mybir.dt.bfloat16
    OP = mybir.AluOpType
    AF = mybir.ActivationFunctionType
    AX = mybir.AxisListType

    with (
        tc.tile_pool(name="const", bufs=1) as constp,
        tc.tile_pool(name="xin", bufs=4) as xinp,
        tc.tile_pool(name="sq", bufs=3) as sqp,
        tc.tile_pool(name="stats", bufs=1) as statsp,
        tc.tile_pool(name="small", bufs=1) as smallp,
        tc.tile_pool(name="psum", bufs=1, space="PSUM") as psump,
        tc.tile_pool(name="dram", bufs=1, space="DRAM") as dramp,
    ):
        # ---- constants (small, on the gpsimd queue) ----
        o23w = constp.tile([128, NBLK, NCH, NP32], bf16)
        nc.gpsimd.dma_start(o23w[:], o23w_d)
        onesw = constp.tile([128, NBLK, NP32], bf16)
        nc.gpsimd.dma_start(onesw[:], onesw_d)
        o23i = constp.tile([NP32, D], f32)
        nc.gpsimd.dma_start(o23i[:], o23i_d)
        l32 = constp.tile([NP32, NP32], f32)
        nc.gpsimd.dma_start(l32[:], l32_d)
        mlt16 = constp.tile([2 * NCORES, NP32], f32)
        nc.gpsimd.dma_start(mlt16[:], mlt16_d)
        sel32 = constp.tile([NP32, 2], f32)
        nc.gpsimd.dma_start(sel32[:], sel32_d)
        ones32 = constp.tile([NP32, 2], f32)
        nc.vector.memset(ones32[:], 1.0)

        # 1/||o2||, 1/||o3|| interleaved on the 32 stats partitions
        sco = smallp.tile([NP32, D], f32)
        so = smallp.tile([NP32, 1], f32)
        nc.scalar.activation(sco[:], o23i[:], AF.Square, accum_out=so[:])
        no = smallp.tile([NP32, 1], f32)
        nc.scalar.activation(no[:], so[:], AF.Sqrt)
        iv32 = smallp.tile([NP32, 1], f32)
        nc.vector.reciprocal(iv32[:], no[:])

        # ---- streamed main loop ----
        # The stationaries place block g's pair at columns (2g, 2g+1), so
        # every block's matmuls land directly on its own PSUM partitions;
        # both [32, 512] PSUM tiles accumulate across the whole loop.
        dots_ps = psump.tile([NP32, RBLK], f32, tag="dots", bufs=1)
        ssq_ps = psump.tile([NP32, RBLK], f32, tag="ssq", bufs=1)

        # xs is xT [D, SH]; tile (p=d-in-chunk, c=chunk, r=row-in-block)
        xv = xs.rearrange("(c p) (g r) -> g p c r", p=128, g=NBLK)
        for g in range(NBLK):
            xt = xinp.tile([128, NCH, RBLK], bf16)
            if g % 2 == 0:
                nc.sync.dma_start(xt[:], xv[g])
            else:
                nc.gpsimd.dma_start(xt[:], xv[g])
            for c in range(NCH):
                nc.tensor.matmul(
                    dots_ps[:], o23w[:, g, c, :], xt[:, c, :],
                    start=(g == 0 and c == 0),
                    stop=(g == NBLK - 1 and c == NCH - 1))
            sq = sqp.tile([128, NCH, RBLK], bf16)
            nc.scalar.activation(sq[:], xt[:], AF.Square)
            for c in range(NCH):
                nc.tensor.matmul(
                    ssq_ps[:], onesw[:, g, :], sq[:, c, :],
                    start=(g == 0 and c == 0),
                    stop=(g == NBLK - 1 and c == NCH - 1))

        # ---- tail: cosines -> exp -> scans -> logs ----
        nrm = statsp.tile([NP32, RBLK], f32)
        nc.scalar.activation(nrm[:], ssq_ps[:], AF.Sqrt)
        rs = statsp.tile([NP32, RBLK], f32)
        nc.vector.reciprocal(rs[:], nrm[:])
        t32 = statsp.tile([NP32, RBLK], f32)
        nc.vector.tensor_tensor(out=t32[:], in0=dots_ps[:], in1=rs[:],
                                op=OP.mult)
        e32 = statsp.tile([NP32, RBLK], f32)
        nc.scalar.activation(e32[:], t32[:], AF.Exp, scale=iv32[:])
        tot32 = smallp.tile([NP32, 1], f32)
        nc.vector.tensor_reduce(out=tot32[:], in_=e32[:], axis=AX.X, op=OP.add)

        # local branch totals -> post the AllGather as early as possible
        tl_ps = psump.tile([2, 1], f32, tag="tail", bufs=2)
        nc.tensor.matmul(tl_ps[:], sel32[:], tot32[:], start=True, stop=True)
        tl = smallp.tile([2, 1], f32)
        nc.vector.tensor_copy(tl[:], tl_ps[:])
        cc_in = dramp.tile([2, 1], f32)
        cc_out = dramp.tile([2 * NCORES, 1], f32, addr_space="Shared")
        nc.sync.dma_start(cc_in[:], tl[:])
        nc.gpsimd.collective_compute(
            "AllGather", OP.bypass, replica_groups=[list(range(NCORES))],
            ins=[cc_in.opt()], outs=[cc_out.opt()])

        # block-local exclusive bases + scan (overlaps the AllGather wait)
        excl_ps = psump.tile([NP32, 1], f32, tag="tail", bufs=2)
        nc.tensor.matmul(excl_ps[:], l32[:], tot32[:], start=True, stop=True)
        basec = smallp.tile([NP32, 1], f32)
        nc.vector.tensor_copy(basec[:], excl_ps[:])
        suf32 = statsp.tile([NP32, RBLK], f32)
        nc.vector.tensor_tensor_scan(
            out=suf32[:], data0=e32[:], data1=e32[:], initial=basec[:],
            op0=OP.add, op1=OP.bypass)

        # consume the AllGather: per-partition cross-core bases
        ag = smallp.tile([2 * NCORES, 1], f32)
        nc.sync.dma_start(ag[:], cc_out[:])
        gb_ps = psump.tile([NP32, 1], f32, tag="tail", bufs=2)
        nc.tensor.matmul(gb_ps[:], mlt16[:], ag[:], start=True, stop=True)
        gb32 = smallp.tile([NP32, 1], f32)
        nc.vector.tensor_copy(gb32[:], gb_ps[:])

        # log-reduction (cross-core base folded into the Ln bias)
        lnscr = statsp.tile([NP32, RBLK], f32)
        la32 = smallp.tile([NP32, 1], f32)
        nc.scalar.activation(lnscr[:], suf32[:], AF.Ln, bias=gb32[:],
                             accum_out=la32[:])
        part_ps = psump.tile([1, 1], f32, tag="tail", bufs=2)
        nc.tensor.matmul(part_ps[:], ones32[:, 0:1], la32[:], start=True,
                         stop=True)

        # per-core outputs: fin[0,0]=tot2, fin[1,0]=tot3, fin[0,1]=partial
        finsb = smallp.tile([2, 2], f32)
        nc.vector.tensor_copy(finsb[:, 0:1], tl[:])
        nc.vector.tensor_copy(finsb[0:1, 1:2], part_ps[:])
        nc.sync.dma_start(fin_out[:], finsb[:])


def build_nc():
    global _compiled_nc
    if _compiled_nc is not None:
        return _compiled_nc
    import concourse.bacc as bacc
    import concourse.mybir as mybir
    from concourse import tile

    f32 = mybir.dt.float32
    bf16 = mybir.dt.bfloat16
    nc = bacc.Bacc("TRN2", target_bir_lowering=False, debug=False,
                   num_devices=NCORES)
    xs = nc.dram_tensor("xs", [D, SH], bf16, kind="ExternalInput")
    o23w = nc.dram_tensor("o23w", [128, NBLK, NCH, NP32], bf16,
                          kind="ExternalInput")
    onesw = nc.dram_tensor("onesw", [128, NBLK, NP32], bf16,
                           kind="ExternalInput")
    o23i = nc.dram_tensor("o23i", [NP32, D], f32, kind="ExternalInput")
    l32 = nc.dram_tensor("l32", [NP32, NP32], f32, kind="ExternalInput")
    mlt16 = nc.dram_tensor("mlt16", [2 * NCORES, NP32], f32,
                           kind="ExternalInput")
    sel32 = nc.dram_tensor("sel32", [NP32, 2], f32, kind="ExternalInput")
    fin = nc.dram_tensor("fin", [2, 2], f32, kind="ExternalOutput")

    with tile.TileContext(nc) as tc:
        _body(tc, mybir, xs.ap(), o23w.ap(), onesw.ap(), o23i.ap(), l32.ap(),
              mlt16.ap(), sel32.ap(), fin.ap())
    nc.compile()
    _compiled_nc = nc
    return nc


def make_in_maps(output1, output2, output3, ranking):
    """Host-side shard: stable sort by descending ranking (matching
    jnp.argsort(-ranking)), feed rows in ascending-rank order so forward
    cumsums on-device are the reference's suffix sums; transposed [D, SH]
    bf16 layout per shard."""
    import ml_dtypes
    bf = ml_dtypes.bfloat16
    ranking = np.asarray(ranking, dtype=np.float32)
    order = np.argsort(-ranking, kind="stable")
    rho = order[::-1]
    xs_full = np.asarray(output1, dtype=np.float32)[rho].astype(bf)
    o2 = np.asarray(output2, dtype=np.float32).reshape(D)
    o3 = np.asarray(output3, dtype=np.float32).reshape(D)
    o23 = np.empty((128, NCH, 2), np.float32)
    o23[:, :, 0] = o2.reshape(NCH, 128).T
    o23[:, :, 1] = o3.reshape(NCH, 128).T
    # per-block stationaries: block g's (o2,o3) pair sits at columns
    # (2g, 2g+1) so its matmuls write PSUM partitions (2g, 2g+1) directly
    o23w = np.zeros((128, NBLK, NCH, NP32), np.float32)
    onesw = np.zeros((128, NBLK, NP32), np.float32)
    for g in range(NBLK):
        o23w[:, g, :, 2 * g : 2 * g + 2] = o23
        onesw[:, g, 2 * g : 2 * g + 2] = 1.0
    o23w = o23w.astype(bf)
    onesw = onesw.astype(bf)
    o23i = np.empty((NP32, D), np.float32)
    o23i[0::2] = o2[None, :]
    o23i[1::2] = o3[None, :]
    # strict-lower-triangular within each branch parity: l32[p', p] = 1
    # iff p' < p and p' == p (mod 2)  ->  excl[2g+b] = sum_{g'<g} tot[2g'+b]
    pidx = np.arange(NP32)
    l32 = ((pidx[:, None] < pidx[None, :])
           & (pidx[:, None] % 2 == pidx[None, :] % 2)).astype(np.float32)
    sel32 = np.stack([(pidx % 2 == 0), (pidx % 2 == 1)], axis=1)
    sel32 = sel32.astype(np.float32)
    in_maps = []
    for c in range(NCORES):
        # mlt16[2c'+b', 2g+b] = 1 iff c' < c and b' == b
        row = np.arange(2 * NCORES)
        mlt16 = ((row[:, None] // 2 < c)
                 & (row[:, None] % 2 == pidx[None, :] % 2)).astype(np.float32)
        in_maps.append({
            "xs": np.ascontiguousarray(xs_full[c * SH : (c + 1) * SH].T),
            "o23w": o23w, "onesw": onesw, "o23i": o23i, "l32": l32,
            "mlt16": mlt16, "sel32": sel32,
        })
    return in_maps


def kernel(output1, output2, output3, ranking):
    from concourse.bass_utils import run_bass_kernel_spmd

    nc = build_nc()
    in_maps = make_in_maps(output1, output2, output3, ranking)
    res = run_bass_kernel_spmd(nc, in_maps, core_ids=list(range(NCORES)))
    fins = [np.asarray(r["fin"], dtype=np.float64) for r in res.results]
    t2 = sum(f[0, 0] for f in fins)
    t3 = sum(f[1, 0] for f in fins)
    parts = sum(f[0, 1] for f in fins)
    loss = N * (np.log(t2) + np.log(t3)) - parts
    return np.asarray(loss, dtype=np.float32).reshape(())


# revision 17
# speedup vs baseline: 2.4944x; 1.0294x over previous
"""Trainium2 Bass kernel for nn_ContrastiveLoss (exp-cosine ranking loss).

Math: sort rows of output1 by descending ranking (stable). With
e_b[i] = exp(cos_sim(x_sorted[i], o_b)) for b in {2,3} and suffix sums
suf_b(i) = sum_{j>=i} e_b[j], the reference loss equals

    loss = N*(log T2 + log T3) - sum_i log suf2(i) - sum_i log suf3(i)

where T_b = suf_b(0) is the global total.  Sharding: host sorts by
ranking (shards are rank-contiguous) and feeds rows in ASCENDING rank
order so forward cumsums on-device are exactly the suffix sums of the
reference order.  Each core gets its 8192-row shard TRANSPOSED
[512, 8192] in bf16; o2/o3 are pre-normalized on the host so no norm
prep or reciprocal is needed on device.

Per 512-row block (16 blocks/core), everything is *streamed* through
the PE array (no transposes, no 128-col weight loads):

  dots:  normalized o23 pair stationary at columns (2j, 2j+1),
         xT chunk [128,512] moving -> row dots land directly on the
         block's own PSUM partitions, accumulated across an 8-block
         half-shard into a [16,512] PSUM tile
  norms: DVE squares the xT tile (bf16, 2 elem/cycle); ones placed the
         same way -> second [16,512] PSUM tile = row |x|^2
  1/|x|: ACT exp(-0.5*ln(ssq)) (avoids the slow DVE reciprocal)

Bulk DMA alternates between the two HWDGE queues (sync + scalar),
keeping the gpsimd queue free for the one tiny AllGather.  The shard is
split into two halves so half A's exp/totals run in the shadow of half
B's streaming and the AllGather posts right after the last matmul; the
per-block scans (seeded by strict-lower-triangular matmuls) and the Ln
table preload hide in the AllGather wait.  Each core outputs
(tot2, tot3, sum-of-logs); the host sums 8 of each and forms
N*(log T2 + log T3) - sum(partials).
"""

import numpy as np

N, D = 65536, 512
NCORES = 8
SH = N // NCORES            # 8192 rows per core
NCH = D // 128              # 4 contraction chunks of 128
RBLK = 512                  # rows per block
NBLK = SH // RBLK           # 16 blocks
HB = NBLK // 2              # 8 blocks per half-shard
NP16 = 2 * HB               # 16 stats partitions per half: (j, b) -> 2j+b

_compiled_nc = None


def _half_tail(nc, mybir, sp, dots_ps, ssq_ps, e16, tot16):
    """ssq/dots [16,512] PSUM -> e = exp(cos) [16,512] and totals [16,1]."""
    AF = mybir.ActivationFunctionType
    OP = mybir.AluOpType
    AX = mybir.AxisListType
    ls = sp.tile([NP16, RBLK], mybir.dt.float32, tag="ls")
    nc.scalar.activation(ls[:], ssq_ps[:], AF.Ln)
    rs = sp.tile([NP16, RBLK], mybir.dt.float32, tag="rs")
    nc.scalar.activation(rs[:], ls[:], AF.Exp, scale=-0.5)
    t16 = sp.tile([NP16, RBLK], mybir.dt.float32, tag="t16")
    nc.vector.tensor_tensor(out=t16[:], in0=dots_ps[:], in1=rs[:], op=OP.mult)
    nc.scalar.activation(e16[:], t16[:], AF.Exp)
    nc.vector.tensor_reduce(out=tot16[:], in_=e16[:], axis=AX.X, op=OP.add)


def _body(tc, mybir, xs, o23w_d, onesw_d, la_d, ma_d, mlt16_d, sel16_d,
          fin_out):
    nc = tc.nc
    f32 = mybir.dt.float32
    bf16 = mybir.dt.bfloat16
    OP = mybir.AluOpType
    AF = mybir.ActivationFunctionType

    with (
        tc.tile_pool(name="const", bufs=1) as constp,
        tc.tile_pool(name="xin", bufs=6) as xinp,
        tc.tile_pool(name="sq", bufs=3) as sqp,
        tc.tile_pool(name="stats", bufs=1) as statsp,
        tc.tile_pool(name="scr", bufs=2) as scrp,
        tc.tile_pool(name="small", bufs=1) as smallp,
        tc.tile_pool(name="psum", bufs=1, space="PSUM") as psump,
        tc.tile_pool(name="dram", bufs=1, space="DRAM") as dramp,
    ):
        # ---- constants (small, on the gpsimd queue) ----
        o23w = constp.tile([128, NBLK, NCH, NP16], bf16)
        nc.gpsimd.dma_start(o23w[:], o23w_d)
        onesw = constp.tile([128, NBLK, NP16], bf16)
        nc.gpsimd.dma_start(onesw[:], onesw_d)
        la = constp.tile([NP16, NP16], f32)
        nc.gpsimd.dma_start(la[:], la_d)
        ma = constp.tile([NP16, NP16], f32)
        nc.gpsimd.dma_start(ma[:], ma_d)
        mlt16 = constp.tile([2 * NCORES, NP16], f32)
        nc.gpsimd.dma_start(mlt16[:], mlt16_d)
        sel16 = constp.tile([NP16, 2], f32)
        nc.gpsimd.dma_start(sel16[:], sel16_d)
        ones16 = constp.tile([NP16, 1], f32)
        nc.vector.memset(ones16[:], 1.0)

        # ---- streamed main loop over two half-shards ----
        dotsA = psump.tile([NP16, RBLK], f32, tag="dotsA", bufs=1)
        ssqA = psump.tile([NP16, RBLK], f32, tag="ssqA", bufs=1)
        dotsB = psump.tile([NP16, RBLK], f32, tag="dotsB", bufs=1)
        ssqB = psump.tile([NP16, RBLK], f32, tag="ssqB", bufs=1)
        eA = statsp.tile([NP16, RBLK], f32)
        totA = smallp.tile([NP16, 1], f32)
        eB = statsp.tile([NP16, RBLK], f32)
        totB = smallp.tile([NP16, 1], f32)

        # xs is xT [D, SH]; tile (p=d-in-chunk, c=chunk, r=row-in-block)
        xv = xs.rearrange("(c p) (g r) -> g p c r", p=128, g=NBLK)
        for g in range(NBLK):
            j = g % HB
            dots_ps, ssq_ps = (dotsA, ssqA) if g < HB else (dotsB, ssqB)
            xt = xinp.tile([128, NCH, RBLK], bf16)
            if g % 2 == 0:
                nc.sync.dma_start(xt[:], xv[g])
            else:
                nc.scalar.dma_start(xt[:], xv[g])
            for c in range(NCH):
                nc.tensor.matmul(
                    dots_ps[:], o23w[:, g, c, :], xt[:, c, :],
                    start=(j == 0 and c == 0),
                    stop=(j == HB - 1 and c == NCH - 1))
            sq = sqp.tile([128, NCH, RBLK], bf16)
            nc.vector.tensor_tensor(out=sq[:], in0=xt[:], in1=xt[:],
                                    op=OP.mult)
            for c in range(NCH):
                nc.tensor.matmul(
                    ssq_ps[:], onesw[:, g, :], sq[:, c, :],
                    start=(j == 0 and c == 0),
                    stop=(j == HB - 1 and c == NCH - 1))
            if g == HB - 1:
                # half A's exp-cosine tail overlaps half B's streaming
                _half_tail(nc, mybir, scrp, dotsA, ssqA, eA, totA)
                exclA_ps = psump.tile([NP16, 1], f32, tag="tail", bufs=2)
                nc.tensor.matmul(exclA_ps[:], la[:], totA[:], start=True,
                                 stop=True)
                basecA = smallp.tile([NP16, 1], f32)
                nc.vector.tensor_copy(basecA[:], exclA_ps[:])
                sufA = statsp.tile([NP16, RBLK], f32)
                nc.vector.tensor_tensor_scan(
                    out=sufA[:], data0=eA[:], data1=eA[:], initial=basecA[:],
                    op0=OP.add, op1=OP.bypass)

        _half_tail(nc, mybir, scrp, dotsB, ssqB, eB, totB)

        # core totals [2,1] -> AllGather, posted as early as possible
        tl_ps = psump.tile([2, 1], f32, tag="tail", bufs=2)
        nc.tensor.matmul(tl_ps[:], sel16[:], totA[:], start=True, stop=False)
        nc.tensor.matmul(tl_ps[:], sel16[:], totB[:], start=False, stop=True)
        tl = smallp.tile([2, 1], f32)
        nc.vector.tensor_copy(tl[:], tl_ps[:])
        cc_in = dramp.tile([2, 1], f32)
        cc_out = dramp.tile([2 * NCORES, 1], f32, addr_space="Shared")
        nc.sync.dma_start(cc_in[:], tl[:])
        nc.gpsimd.collective_compute(
            "AllGather", OP.bypass, replica_groups=[list(range(NCORES))],
            ins=[cc_in.opt()], outs=[cc_out.opt()])

        # overlap the AllGather wait: half-B bases + scan, Ln table preload
        exclB_ps = psump.tile([NP16, 1], f32, tag="tail", bufs=2)
        nc.tensor.matmul(exclB_ps[:], ma[:], totA[:], start=True, stop=False)
        nc.tensor.matmul(exclB_ps[:], la[:], totB[:], start=False, stop=True)
        basecB = smallp.tile([NP16, 1], f32)
        nc.vector.tensor_copy(basecB[:], exclB_ps[:])
        sufB = statsp.tile([NP16, RBLK], f32)
        nc.vector.tensor_tensor_scan(
            out=sufB[:], data0=eB[:], data1=eB[:], initial=basecB[:],
            op0=OP.add, op1=OP.bypass)
        lnwarm = smallp.tile([NP16, 1], f32)
        nc.scalar.activation(lnwarm[:], totA[:], AF.Ln)

        # consume the AllGather: per-partition cross-core bases
        ag = smallp.tile([2 * NCORES, 1], f32)
        nc.sync.dma_start(ag[:], cc_out[:])
        gb_ps = psump.tile([NP16, 1], f32, tag="tail", bufs=2)
        nc.tensor.matmul(gb_ps[:], mlt16[:], ag[:], start=True, stop=True)
        gb16 = smallp.tile([NP16, 1], f32)
        nc.vector.tensor_copy(gb16[:], gb_ps[:])

        # log-reduction (cross-core base folded into the Ln bias)
        lnA = scrp.tile([NP16, RBLK], f32, tag="ls")
        laA = smallp.tile([NP16, 1], f32)
        nc.scalar.activation(lnA[:], sufA[:], AF.Ln, bias=gb16[:],
                             accum_out=laA[:])
        lnB = scrp.tile([NP16, RBLK], f32, tag="ls")
        laB = smallp.tile([NP16, 1], f32)
        nc.scalar.activation(lnB[:], sufB[:], AF.Ln, bias=gb16[:],
                             accum_out=laB[:])
        part_ps = psump.tile([1, 1], f32, tag="tail", bufs=2)
        nc.tensor.matmul(part_ps[:], ones16[:], laA[:], start=True, stop=False)
        nc.tensor.matmul(part_ps[:], ones16[:], laB[:], start=False, stop=True)

        # per-core outputs: fin[0,0]=tot2, fin[1,0]=tot3, fin[0,1]=partial
        finsb = smallp.tile([2, 2], f32)
        nc.vector.tensor_copy(finsb[:, 0:1], tl[:])
        nc.vector.tensor_copy(finsb[0:1, 1:2], part_ps[:])
        nc.sync.dma_start(fin_out[:], finsb[:])


def build_nc():
    global _compiled_nc
    if _compiled_nc is not None:
        return _compiled_nc
    import concourse.bacc as bacc
    import concourse.mybir as mybir
    from concourse import tile

    f32 = mybir.dt.float32
    bf16 = mybir.dt.bfloat16
    nc = bacc.Bacc("TRN2", target_bir_lowering=False, debug=False,
                   num_devices=NCORES)
    xs = nc.dram_tensor("xs", [D, SH], bf16, kind="ExternalInput")
    o23w = nc.dram_tensor("o23w", [128, NBLK, NCH, NP16], bf16,
                          kind="ExternalInput")
    onesw = nc.dram_tensor("onesw", [128, NBLK, NP16], bf16,
                           kind="ExternalInput")
    la = nc.dram_tensor("la", [NP16, NP16], f32, kind="ExternalInput")
    ma = nc.dram_tensor("ma", [NP16, NP16], f32, kind="ExternalInput")
    mlt16 = nc.dram_tensor("mlt16", [2 * NCORES, NP16], f32,
                           kind="ExternalInput")
    sel16 = nc.dram_tensor("sel16", [NP16, 2], f32, kind="ExternalInput")
    fin = nc.dram_tensor("fin", [2, 2], f32, kind="ExternalOutput")

    with tile.TileContext(nc) as tc:
        _body(tc, mybir, xs.ap(), o23w.ap(), onesw.ap(), la.ap(), ma.ap(),
              mlt16.ap(), sel16.ap(), fin.ap())
    nc.compile()
    _compiled_nc = nc
    return nc


def make_in_maps(output1, output2, output3, ranking):
    """Host-side shard: stable sort by descending ranking (matching
    jnp.argsort(-ranking)), feed rows in ascending-rank order so forward
    cumsums on-device are the reference's suffix sums; transposed [D, SH]
    bf16 layout per shard; o2/o3 pre-normalized."""
    import ml_dtypes
    bf = ml_dtypes.bfloat16
    ranking = np.asarray(ranking, dtype=np.float32)
    order = np.argsort(-ranking, kind="stable")
    rho = order[::-1]
    xs_full = np.asarray(output1, dtype=np.float32)[rho].astype(bf)
    o2 = np.asarray(output2, dtype=np.float32).reshape(D)
    o3 = np.asarray(output3, dtype=np.float32).reshape(D)
    o2 = o2 / np.linalg.norm(o2)
    o3 = o3 / np.linalg.norm(o3)
    o23 = np.empty((128, NCH, 2), np.float32)
    o23[:, :, 0] = o2.reshape(NCH, 128).T
    o23[:, :, 1] = o3.reshape(NCH, 128).T
    # per-block stationaries: block g's (o2,o3) pair sits at columns
    # (2j, 2j+1), j = g mod 8, so its matmuls write its own PSUM partitions
    o23w = np.zeros((128, NBLK, NCH, NP16), np.float32)
    onesw = np.zeros((128, NBLK, NP16), np.float32)
    for g in range(NBLK):
        j = g % HB
        o23w[:, g, :, 2 * j : 2 * j + 2] = o23
        onesw[:, g, 2 * j : 2 * j + 2] = 1.0
    o23w = o23w.astype(bf)
    onesw = onesw.astype(bf)
    pidx = np.arange(NP16)
    par_match = pidx[:, None] % 2 == pidx[None, :] % 2
    la = ((pidx[:, None] < pidx[None, :]) & par_match).astype(np.float32)
    ma = par_match.astype(np.float32)
    sel16 = np.stack([(pidx % 2 == 0), (pidx % 2 == 1)], axis=1)
    sel16 = sel16.astype(np.float32)
    in_maps = []
    for c in range(NCORES):
        row = np.arange(2 * NCORES)
        mlt16 = ((row[:, None] // 2 < c)
                 & (row[:, None] % 2 == pidx[None, :] % 2)).astype(np.float32)
        in_maps.append({
            "xs": np.ascontiguousarray(xs_full[c * SH : (c + 1) * SH].T),
            "o23w": o23w, "onesw": onesw, "la": la, "ma": ma,
            "mlt16": mlt16, "sel16": sel16,
        })
    return in_maps


def kernel(output1, output2, output3, ranking):
    from concourse.bass_utils import run_bass_kernel_spmd

    nc = build_nc()
    in_maps = make_in_maps(output1, output2, output3, ranking)
    res = run_bass_kernel_spmd(nc, in_maps, core_ids=list(range(NCORES)))
    fins = [np.asarray(r["fin"], dtype=np.float64) for r in res.results]
    t2 = sum(f[0, 0] for f in fins)
    t3 = sum(f[1, 0] for f in fins)
    parts = sum(f[0, 1] for f in fins)
    loss = N * (np.log(t2) + np.log(t3)) - parts
    return np.asarray(loss, dtype=np.float32).reshape(())


# revision 19
# speedup vs baseline: 2.6161x; 1.0488x over previous
"""Trainium2 Bass kernel for nn_ContrastiveLoss (exp-cosine ranking loss).

Math: sort rows of output1 by descending ranking (stable). With
e_b[i] = exp(cos_sim(x_sorted[i], o_b)) for b in {2,3} and suffix sums
suf_b(i) = sum_{j>=i} e_b[j], the reference loss equals

    loss = N*(log T2 + log T3) - sum_i log suf2(i) - sum_i log suf3(i)

where T_b = suf_b(0) is the global total.  Sharding: host sorts by
ranking (shards are rank-contiguous) and feeds rows in ASCENDING rank
order so forward cumsums on-device are exactly the suffix sums of the
reference order.  Each core gets its 8192-row shard TRANSPOSED
[512, 8192] in bf16; o2/o3 are pre-normalized on the host so no norm
prep or reciprocal is needed on device.

Per 512-row block (16 blocks/core), everything is *streamed* through
the PE array (no transposes, no 128-col weight loads):

  dots:  normalized o23 pair stationary at columns (2j, 2j+1),
         xT chunk [128,512] moving -> row dots land directly on the
         block's own PSUM partitions, accumulated across an 8-block
         half-shard into a [16,512] PSUM tile
  norms: DVE squares the xT tile (bf16, 2 elem/cycle); ones placed the
         same way -> second [16,512] PSUM tile = row |x|^2
  1/|x|: ACT exp(-0.5*ln(ssq)) (avoids the slow DVE reciprocal)

Bulk DMA alternates between the two HWDGE queues (sync + scalar),
keeping the gpsimd queue free for the one tiny AllGather.  The shard is
split into two halves so half A's exp/totals run in the shadow of half
B's streaming and the AllGather posts right after the last matmul; the
per-block scans (seeded by strict-lower-triangular matmuls) and the Ln
table preload hide in the AllGather wait.  Each core outputs
(tot2, tot3, sum-of-logs); the host sums 8 of each and forms
N*(log T2 + log T3) - sum(partials).
"""

import numpy as np

N, D = 65536, 512
NCORES = 8
SH = N // NCORES            # 8192 rows per core
NCH = D // 128              # 4 contraction chunks of 128
RBLK = 512                  # rows per block
NBLK = SH // RBLK           # 16 blocks
HB = NBLK // 2              # 8 blocks per half-shard
NP16 = 2 * HB               # 16 stats partitions per half: (j, b) -> 2j+b

_compiled_nc = None


def _half_tail(nc, mybir, sp, dots_ps, ssq_ps, e16, tot16):
    """ssq/dots [16,512] PSUM -> e = exp(cos) [16,512] and totals [16,1]."""
    AF = mybir.ActivationFunctionType
    OP = mybir.AluOpType
    AX = mybir.AxisListType
    ls = sp.tile([NP16, RBLK], mybir.dt.float32, tag="ls")
    nc.scalar.activation(ls[:], ssq_ps[:], AF.Ln)
    rs = sp.tile([NP16, RBLK], mybir.dt.float32, tag="rs")
    nc.scalar.activation(rs[:], ls[:], AF.Exp, scale=-0.5)
    t16 = sp.tile([NP16, RBLK], mybir.dt.float32, tag="t16")
    nc.vector.tensor_tensor(out=t16[:], in0=dots_ps[:], in1=rs[:], op=OP.mult)
    nc.scalar.activation(e16[:], t16[:], AF.Exp)
    nc.vector.tensor_reduce(out=tot16[:], in_=e16[:], axis=AX.X, op=OP.add)


def _body(tc, mybir, xs, o23w_d, onesw_d, la_d, ma_d, mlt16_d, sel16_d,
          fin_out):
    nc = tc.nc
    f32 = mybir.dt.float32
    bf16 = mybir.dt.bfloat16
    OP = mybir.AluOpType
    AF = mybir.ActivationFunctionType

    with (
        tc.tile_pool(name="const", bufs=1) as constp,
        tc.tile_pool(name="xin", bufs=6) as xinp,
        tc.tile_pool(name="sq", bufs=3) as sqp,
        tc.tile_pool(name="stats", bufs=1) as statsp,
        tc.tile_pool(name="scr", bufs=2) as scrp,
        tc.tile_pool(name="small", bufs=1) as smallp,
        tc.tile_pool(name="psum", bufs=1, space="PSUM") as psump,
        tc.tile_pool(name="dram", bufs=1, space="DRAM") as dramp,
    ):
        # ---- PE warm-up: pull the HAM clock gate to 8/8 before the real
        # stream arrives (first xt DMA lands ~10us in)
        wsrc = constp.tile([128, RBLK], bf16)
        nc.vector.memset(wsrc[:], 0.0)
        warm_ps = psump.tile([NP16, RBLK], f32, tag="warm", bufs=1)
        for _ in range(12):
            nc.tensor.matmul(warm_ps[:], wsrc[:, 0:NP16], wsrc[:],
                             start=True, stop=True)

        # ---- constants (small, on the gpsimd queue) ----
        o23w = constp.tile([128, NBLK, NCH, NP16], bf16)
        nc.gpsimd.dma_start(o23w[:], o23w_d)
        onesw = constp.tile([128, NBLK, NP16], bf16)
        nc.gpsimd.dma_start(onesw[:], onesw_d)
        la = constp.tile([NP16, NP16], f32)
        nc.gpsimd.dma_start(la[:], la_d)
        ma = constp.tile([NP16, NP16], f32)
        nc.gpsimd.dma_start(ma[:], ma_d)
        mlt16 = constp.tile([2 * NCORES, NP16], f32)
        nc.gpsimd.dma_start(mlt16[:], mlt16_d)
        sel16 = constp.tile([NP16, 2], f32)
        nc.gpsimd.dma_start(sel16[:], sel16_d)
        ones16 = constp.tile([NP16, 1], f32)
        nc.vector.memset(ones16[:], 1.0)

        # dummy AllGather: pays the CC stream's first-op sync/setup cost
        # in the shadow of the main loop so the real one starts promptly
        cc0_in = dramp.tile([1, 1], f32)
        cc0_out = dramp.tile([NCORES, 1], f32, addr_space="Shared")
        nc.sync.dma_start(cc0_in[:], ones16[0:1, :])
        nc.gpsimd.collective_compute(
            "AllGather", OP.bypass, replica_groups=[list(range(NCORES))],
            ins=[cc0_in.opt()], outs=[cc0_out.opt()])

        # ---- streamed main loop over two half-shards ----
        dotsA = psump.tile([NP16, RBLK], f32, tag="dotsA", bufs=1)
        ssqA = psump.tile([NP16, RBLK], f32, tag="ssqA", bufs=1)
        dotsB = psump.tile([NP16, RBLK], f32, tag="dotsB", bufs=1)
        ssqB = psump.tile([NP16, RBLK], f32, tag="ssqB", bufs=1)
        eA = statsp.tile([NP16, RBLK], f32)
        totA = smallp.tile([NP16, 1], f32)
        eB = statsp.tile([NP16, RBLK], f32)
        totB = smallp.tile([NP16, 1], f32)

        # xs is xT [D, SH]; tile (p=d-in-chunk, c=chunk, r=row-in-block)
        xv = xs.rearrange("(c p) (g r) -> g p c r", p=128, g=NBLK)
        for g in range(NBLK):
            j = g % HB
            dots_ps, ssq_ps = (dotsA, ssqA) if g < HB else (dotsB, ssqB)
            xt = xinp.tile([128, NCH, RBLK], bf16)
            if g % 2 == 0:
                nc.sync.dma_start(xt[:], xv[g])
            else:
                nc.scalar.dma_start(xt[:], xv[g])
            for c in range(NCH):
                nc.tensor.matmul(
                    dots_ps[:], o23w[:, g, c, :], xt[:, c, :],
                    start=(j == 0 and c == 0),
                    stop=(j == HB - 1 and c == NCH - 1))
            sq = sqp.tile([128, NCH, RBLK], bf16)
            nc.vector.tensor_tensor(out=sq[:], in0=xt[:], in1=xt[:],
                                    op=OP.mult)
            for c in range(NCH):
                nc.tensor.matmul(
                    ssq_ps[:], onesw[:, g, :], sq[:, c, :],
                    start=(j == 0 and c == 0),
                    stop=(j == HB - 1 and c == NCH - 1))
            if g == HB - 1:
                # half A's exp-cosine tail overlaps half B's streaming
                _half_tail(nc, mybir, scrp, dotsA, ssqA, eA, totA)
                exclA_ps = psump.tile([NP16, 1], f32, tag="tail", bufs=2)
                nc.tensor.matmul(exclA_ps[:], la[:], totA[:], start=True,
                                 stop=True)
                basecA = smallp.tile([NP16, 1], f32)
                nc.vector.tensor_copy(basecA[:], exclA_ps[:])
                sufA = statsp.tile([NP16, RBLK], f32)
                nc.vector.tensor_tensor_scan(
                    out=sufA[:], data0=eA[:], data1=eA[:], initial=basecA[:],
                    op0=OP.add, op1=OP.bypass)

        _half_tail(nc, mybir, scrp, dotsB, ssqB, eB, totB)

        # core totals [2,1] -> AllGather, posted as early as possible
        tl_ps = psump.tile([2, 1], f32, tag="tail", bufs=2)
        nc.tensor.matmul(tl_ps[:], sel16[:], totA[:], start=True, stop=False)
        nc.tensor.matmul(tl_ps[:], sel16[:], totB[:], start=False, stop=True)
        tl = smallp.tile([2, 1], f32)
        nc.vector.tensor_copy(tl[:], tl_ps[:])
        cc_in = dramp.tile([2, 1], f32)
        cc_out = dramp.tile([2 * NCORES, 1], f32, addr_space="Shared")
        nc.sync.dma_start(cc_in[:], tl[:])
        nc.gpsimd.collective_compute(
            "AllGather", OP.bypass, replica_groups=[list(range(NCORES))],
            ins=[cc_in.opt()], outs=[cc_out.opt()])

        # overlap the AllGather wait: half-B bases + scan, Ln table preload
        exclB_ps = psump.tile([NP16, 1], f32, tag="tail", bufs=2)
        nc.tensor.matmul(exclB_ps[:], ma[:], totA[:], start=True, stop=False)
        nc.tensor.matmul(exclB_ps[:], la[:], totB[:], start=False, stop=True)
        basecB = smallp.tile([NP16, 1], f32)
        nc.vector.tensor_copy(basecB[:], exclB_ps[:])
        sufB = statsp.tile([NP16, RBLK], f32)
        nc.vector.tensor_tensor_scan(
            out=sufB[:], data0=eB[:], data1=eB[:], initial=basecB[:],
            op0=OP.add, op1=OP.bypass)
        lnwarm = smallp.tile([NP16, 1], f32)
        nc.scalar.activation(lnwarm[:], totA[:], AF.Ln)

        # consume the AllGather: per-partition cross-core bases
        ag = smallp.tile([2 * NCORES, 1], f32)
        nc.sync.dma_start(ag[:], cc_out[:])
        gb_ps = psump.tile([NP16, 1], f32, tag="tail", bufs=2)
        nc.tensor.matmul(gb_ps[:], mlt16[:], ag[:], start=True, stop=True)
        gb16 = smallp.tile([NP16, 1], f32)
        nc.vector.tensor_copy(gb16[:], gb_ps[:])

        # log-reduction (cross-core base folded into the Ln bias)
        lnA = scrp.tile([NP16, RBLK], f32, tag="ls")
        laA = smallp.tile([NP16, 1], f32)
        nc.scalar.activation(lnA[:], sufA[:], AF.Ln, bias=gb16[:],
                             accum_out=laA[:])
        lnB = scrp.tile([NP16, RBLK], f32, tag="ls")
        laB = smallp.tile([NP16, 1], f32)
        nc.scalar.activation(lnB[:], sufB[:], AF.Ln, bias=gb16[:],
                             accum_out=laB[:])
        part_ps = psump.tile([1, 1], f32, tag="tail", bufs=2)
        nc.tensor.matmul(part_ps[:], ones16[:], laA[:], start=True, stop=False)
        nc.tensor.matmul(part_ps[:], ones16[:], laB[:], start=False, stop=True)

        # per-core outputs: fin[0,0]=tot2, fin[1,0]=tot3, fin[0,1]=partial
        finsb = smallp.tile([2, 2], f32)
        nc.vector.tensor_copy(finsb[:, 0:1], tl[:])
        nc.vector.tensor_copy(finsb[0:1, 1:2], part_ps[:])
        nc.sync.dma_start(fin_out[:], finsb[:])


def build_nc():
    global _compiled_nc
    if _compiled_nc is not None:
        return _compiled_nc
    import concourse.bacc as bacc
    import concourse.mybir as mybir
    from concourse import tile

    f32 = mybir.dt.float32
    bf16 = mybir.dt.bfloat16
    nc = bacc.Bacc("TRN2", target_bir_lowering=False, debug=False,
                   num_devices=NCORES)
    xs = nc.dram_tensor("xs", [D, SH], bf16, kind="ExternalInput")
    o23w = nc.dram_tensor("o23w", [128, NBLK, NCH, NP16], bf16,
                          kind="ExternalInput")
    onesw = nc.dram_tensor("onesw", [128, NBLK, NP16], bf16,
                           kind="ExternalInput")
    la = nc.dram_tensor("la", [NP16, NP16], f32, kind="ExternalInput")
    ma = nc.dram_tensor("ma", [NP16, NP16], f32, kind="ExternalInput")
    mlt16 = nc.dram_tensor("mlt16", [2 * NCORES, NP16], f32,
                           kind="ExternalInput")
    sel16 = nc.dram_tensor("sel16", [NP16, 2], f32, kind="ExternalInput")
    fin = nc.dram_tensor("fin", [2, 2], f32, kind="ExternalOutput")

    with tile.TileContext(nc) as tc:
        _body(tc, mybir, xs.ap(), o23w.ap(), onesw.ap(), la.ap(), ma.ap(),
              mlt16.ap(), sel16.ap(), fin.ap())
    nc.compile()
    _compiled_nc = nc
    return nc


def make_in_maps(output1, output2, output3, ranking):
    """Host-side shard: stable sort by descending ranking (matching
    jnp.argsort(-ranking)), feed rows in ascending-rank order so forward
    cumsums on-device are the reference's suffix sums; transposed [D, SH]
    bf16 layout per shard; o2/o3 pre-normalized."""
    import ml_dtypes
    bf = ml_dtypes.bfloat16
    ranking = np.asarray(ranking, dtype=np.float32)
    order = np.argsort(-ranking, kind="stable")
    rho = order[::-1]
    xs_full = np.asarray(output1, dtype=np.float32)[rho].astype(bf)
    o2 = np.asarray(output2, dtype=np.float32).reshape(D)
    o3 = np.asarray(output3, dtype=np.float32).reshape(D)
    o2 = o2 / np.linalg.norm(o2)
    o3 = o3 / np.linalg.norm(o3)
    o23 = np.empty((128, NCH, 2), np.float32)
    o23[:, :, 0] = o2.reshape(NCH, 128).T
    o23[:, :, 1] = o3.reshape(NCH, 128).T
    # per-block stationaries: block g's (o2,o3) pair sits at columns
    # (2j, 2j+1), j = g mod 8, so its matmuls write its own PSUM partitions
    o23w = np.zeros((128, NBLK, NCH, NP16), np.float32)
    onesw = np.zeros((128, NBLK, NP16), np.float32)
    for g in range(NBLK):
        j = g % HB
        o23w[:, g, :, 2 * j : 2 * j + 2] = o23
        onesw[:, g, 2 * j : 2 * j + 2] = 1.0
    o23w = o23w.astype(bf)
    onesw = onesw.astype(bf)
    pidx = np.arange(NP16)
    par_match = pidx[:, None] % 2 == pidx[None, :] % 2
    la = ((pidx[:, None] < pidx[None, :]) & par_match).astype(np.float32)
    ma = par_match.astype(np.float32)
    sel16 = np.stack([(pidx % 2 == 0), (pidx % 2 == 1)], axis=1)
    sel16 = sel16.astype(np.float32)
    in_maps = []
    for c in range(NCORES):
        row = np.arange(2 * NCORES)
        mlt16 = ((row[:, None] // 2 < c)
                 & (row[:, None] % 2 == pidx[None, :] % 2)).astype(np.float32)
        in_maps.append({
            "xs": np.ascontiguousarray(xs_full[c * SH : (c + 1) * SH].T),
            "o23w": o23w, "onesw": onesw, "la": la, "ma": ma,
            "mlt16": mlt16, "sel16": sel16,
        })
    return in_maps


def kernel(output1, output2, output3, ranking):
    from concourse.bass_utils import run_bass_kernel_spmd

    nc = build_nc()
    in_maps = make_in_maps(output1, output2, output3, ranking)
    res = run_bass_kernel_spmd(nc, in_maps, core_ids=list(range(NCORES)))
    fins = [np.asarray(r["fin"], dtype=np.float64) for r in res.results]
    t2 = sum(f[0, 0] for f in fins)
    t3 = sum(f[1, 0] for f in fins)
    parts = sum(f[0, 1] for f in fins)
    loss = N * (np.log(t2) + np.log(t3)) - parts
    return np.asarray(loss, dtype=np.float32).reshape(())
